# revision 39
# baseline (speedup 1.0000x reference)
"""MatchingNet head (cosine-sim kNN aggregation) on 8 trn2 NeuronCores.

Reference computation:
    sim[m, n] = <fX[m], gS[n]> / max(||fX[m]|| * ||gS[n]||, 1e-8)
    out[m, c] = sum_n sim[m, n] * onehot(trainTarget)[n, c]

Exact algebraic reassociation (the eps guard never binds for D=1024 randn
rows, whose norms concentrate around 32):
    A = gS.T @ (onehot / ||gS||)          # [D, C]
    out = diag(1/||fX||) @ (fX @ A)        # [M, C]

Two SPMD launches (no collectives under this runtime):
  Phase 1: gS sharded row-wise; core i computes the partial
           A_i.T = (onehot_i / ||gS_i||).T @ gS_i  over its 512 supports.
           Host sums the eight [64, 1024] partials and retiles for phase 2.
  Phase 2: fX sharded row-wise; each core computes its transposed slab
           outT = A.T @ fX_i.T scaled by 1/||fX||; host transposes back.

All device inputs are bf16 (harness tolerance is 2e-2; measured end-to-end
error of the bf16 pipeline is ~3e-3). Matmuls accumulate in fp32 PSUM.
Query norms: squares on DVE (bf16, off the DMA critical path), reduced
over the contraction dim by ones-matmuls into a PSUM accumulator that
yields ||fX||^2 replicated across the 64 output partitions, so the final
1/||fX|| scale is a plain tensor_mul against the PSUM output slab --
no single-partition ops, no PE transposes, no PE norm extracts.
"""

import numpy as np
import ml_dtypes
from contextlib import ExitStack

import concourse.bass as bass  # noqa: F401
import concourse.bass_isa as bass_isa  # noqa: F401
import concourse.tile as tile
import concourse.mybir as mybir
from concourse import bacc, bass2jax
from concourse.bass_utils import run_bass_kernel_spmd
from concourse.tile import add_dep_helper

N, D, C, M = 4096, 1024, 64, 8192
NCORES = 8
NS = N // NCORES   # 512 supports per core (phase 1)
MS = M // NCORES   # 1024 queries per core (phase 2)
P = 128
NT = NS // P       # 4 support tiles per core
DC = D // P        # 8 d-chunks of 128
HB = 512           # PSUM bank width in fp32
F32 = mybir.dt.float32
BF16 = mybir.dt.bfloat16
AF = mybir.ActivationFunctionType
MULT = mybir.AluOpType.mult
BF = ml_dtypes.bfloat16

_CACHE = {}


def _build_phase1():
    nc = bacc.Bacc(
        "TRN2", target_bir_lowering=False, debug=False, num_devices=NCORES
    )
    gs = nc.dram_tensor("gs", [P, NT, D], BF16, kind="ExternalInput").ap()
    oh = nc.dram_tensor("oh", [P, NT, C], BF16, kind="ExternalInput").ap()
    atp = nc.dram_tensor("atp", [C, D], BF16, kind="ExternalOutput").ap()

    with tile.TileContext(nc) as tc, ExitStack() as ctx:
        const_pool = ctx.enter_context(tc.tile_pool(name="const", bufs=1))
        sq_pool = ctx.enter_context(tc.tile_pool(name="sqp", bufs=2))
        st_pool = ctx.enter_context(tc.tile_pool(name="stp", bufs=4))
        w_pool = ctx.enter_context(tc.tile_pool(name="wp", bufs=3))
        os_pool = ctx.enter_context(tc.tile_pool(name="osp", bufs=2))
        psA = ctx.enter_context(tc.tile_pool(name="psA", bufs=1, space="PSUM"))
        psW = ctx.enter_context(tc.tile_pool(name="psW", bufs=1, space="PSUM"))

        # DMA dispatches first: oh on ACT's HWDGE ring, gs tiles split
        # between Sync (HWDGE) and GpSimd (SWDGE) so descriptor generation
        # for consecutive tiles overlaps instead of serializing.
        # All three descriptor lanes (2x HWDGE + SWDGE) in consumption
        # order: per-lane transfer cadence is ~2.3us (bandwidth share +
        # completion receipt), so three lanes interleave tile arrivals at
        # ~0.8us effective cadence. oh (64 KB) rides SWDGE first.
        gs_sb = const_pool.tile([P, NT * D], BF16, tag="gs")
        oh_sb = const_pool.tile([P, NT * C], BF16, tag="oh")
        nc.sync.dma_start(gs_sb[:, 0:D], gs[:, 0, :])
        nc.scalar.dma_start(gs_sb[:, D:2 * D], gs[:, 1, :])
        nc.gpsimd.dma_start(
            oh_sb[:].rearrange("p (t c) -> p t c", t=NT), oh[:, :, :]
        )
        nc.sync.dma_start(gs_sb[:, 2 * D:3 * D], gs[:, 2, :])
        nc.scalar.dma_start(gs_sb[:, 3 * D:4 * D], gs[:, 3, :])

        # Hoist the sqrt table-set load off the critical path.
        dumm = st_pool.tile([P, 1], F32, tag="dumm")
        nc.vector.memset(dumm[:], 1.0)
        dumm2 = st_pool.tile([P, 1], F32, tag="dumm2")
        nc.scalar.activation(dumm2[:], dumm[:], AF.Sqrt)

        # PE warm-up: garbage matmuls while the PE would otherwise idle
        # waiting for the first gs tile. Trips the HAM activity monitor to
        # K=8/8 so the real matmuls run at 2.4 GHz instead of 1.2; sized so
        # it ends roughly when the first weight tile becomes ready (a gap
        # >~3.4us would re-throttle).
        wz = const_pool.tile([P, HB], BF16, tag="wz")
        nc.vector.memset(wz[:], 1.0)
        pw = psW.tile([P, HB], F32, tag="pw")
        # PE warm-up sized to end just as the first weight tile is ready:
        # longer would block the real matmuls behind the PE FIFO, shorter
        # leaves a >3.4us idle gap that re-throttles the HAM clock gate.
        NWARM = 10
        for i in range(NWARM):
            nc.tensor.matmul(
                pw[:], wz[:, :P], wz[:],
                start=(i == 0), stop=(i == NWARM - 1),
            )

        # Per-tile norm chain -> weights -> matmuls, interleaved so each
        # tile's chain starts as soon as its data lands. Squares split
        # DVE (t0, t2) / ACT (t1, t3) so two tiles process concurrently.
        pa = psA.tile([C, 2 * HB], F32, tag="pa")
        sqrt_insts = []
        act_sq_insts = []
        for t in range(NT):
            seg = gs_sb[:, t * D:(t + 1) * D]
            gsq = st_pool.tile([P, 1], F32, tag=f"gsq{t}", name=f"gsq{t}")
            if t % 2 == 0:
                sqd = sq_pool.tile([P, D], BF16, tag="sqv")
                nc.vector.scalar_tensor_tensor(
                    out=sqd[:], in0=seg, scalar=1.0, in1=seg,
                    op0=MULT, op1=MULT, accum_out=gsq[:],
                )
            else:
                sqd = sq_pool.tile([P, D], F32, tag="sqa")
                si = nc.scalar.activation(
                    sqd[:], seg, AF.Square, accum_out=gsq[:]
                )
                act_sq_insts.append(si)
            gnorm = st_pool.tile([P, 1], F32, tag="gn")
            qi = nc.scalar.activation(gnorm[:], gsq[:], AF.Sqrt)
            sqrt_insts.append(qi)
            grinv = st_pool.tile([P, 1], F32, tag="gr")
            nc.vector.reciprocal_approx_fast(grinv[:], gnorm[:])
            wt = w_pool.tile([P, C], BF16, tag="w")
            nc.vector.tensor_scalar_mul(
                wt[:], oh_sb[:, t * C:(t + 1) * C], grinv[:]
            )
            for h in range(2):
                nc.tensor.matmul(
                    pa[:, h * HB:(h + 1) * HB],
                    wt[:],
                    seg[:, h * HB:(h + 1) * HB],
                    start=(t == 0),
                    stop=(t == NT - 1),
                )
        # Keep ACT from running the t3 square ahead of the earlier tiles'
        # sqrts (observed scheduler failure mode: all ACT squares first,
        # every weight chain stalls).
        if len(act_sq_insts) > 1:
            for qi in sqrt_insts[:2]:
                add_dep_helper(
                    act_sq_insts[1].ins, qi.ins, sync=False,
                    reason="norm sqrts before the last ACT square",
                )


        o = os_pool.tile([C, 2 * HB], BF16, tag="o")
        nc.vector.tensor_copy(o[:], pa[:])
        for h in range(2):
            (nc.sync, nc.scalar)[h].dma_start(
                atp[:, h * HB:(h + 1) * HB], o[:, h * HB:(h + 1) * HB]
            )

    nc.compile()
    return nc


def _build_phase2():
    nc = bacc.Bacc(
        "TRN2", target_bir_lowering=False, debug=False, num_devices=NCORES
    )
    a = nc.dram_tensor("a", [P, DC, C], BF16, kind="ExternalInput").ap()
    fxt = nc.dram_tensor("fxt", [P, DC, MS], BF16, kind="ExternalInput").ap()
    ot = nc.dram_tensor("ot", [C, MS], BF16, kind="ExternalOutput").ap()

    with tile.TileContext(nc) as tc, ExitStack() as ctx:
        const_pool = ctx.enter_context(tc.tile_pool(name="const", bufs=1))
        sq_pool = ctx.enter_context(tc.tile_pool(name="sqp", bufs=3))
        st_pool = ctx.enter_context(tc.tile_pool(name="stp", bufs=2))
        nm_pool = ctx.enter_context(tc.tile_pool(name="nmp", bufs=2))
        os_pool = ctx.enter_context(tc.tile_pool(name="osp", bufs=2))
        psO = ctx.enter_context(tc.tile_pool(name="psO", bufs=1, space="PSUM"))
        psF = ctx.enter_context(tc.tile_pool(name="psF", bufs=1, space="PSUM"))
        psW = ctx.enter_context(tc.tile_pool(name="psW", bufs=1, space="PSUM"))

        # A first (needed by the first matmul), on ACT's ring; fxt chunks
        # round-robin over three descriptor-generation engines. Chunk 0 is
        # loaded as two halves so the first matmul can start earlier.
        # The first matmul needs only A's k=0 slab (16 KB) and the first
        # piece of chunk 0 -- load those as small early transfers so they
        # complete ahead of the round-robin smear of the full stream.
        a_sb = const_pool.tile([P, DC * C], BF16, tag="a")
        nc.scalar.dma_start(a_sb[:, 0:C], a[:, 0, :])
        nc.scalar.dma_start(
            a_sb[:, C:].rearrange("p (k c) -> p k c", k=DC - 1),
            a[:, 1:, :],
        )
        # Per-chunk transfers round-robin over the three descriptor lanes;
        # chunk 0 in pieces on sync (a 64 KB quarter first) so the first
        # matmul starts early.
        QB = HB // 2
        fxt_sb = const_pool.tile([P, DC * MS], BF16, tag="fxt")
        nc.sync.dma_start(fxt_sb[:, 0:QB], fxt[:, 0, 0:QB])
        nc.sync.dma_start(fxt_sb[:, QB:HB], fxt[:, 0, QB:HB])
        nc.sync.dma_start(fxt_sb[:, HB:MS], fxt[:, 0, HB:MS])
        for k in range(1, DC):
            eng = (nc.sync, nc.gpsimd, nc.scalar)[k % 3]
            eng.dma_start(
                fxt_sb[:, k * MS:(k + 1) * MS], fxt[:, k, :]
            )

        wz = const_pool.tile([P, HB], BF16, tag="wz")
        nc.vector.memset(wz[:], 1.0)
        ones64 = const_pool.tile([P, C], BF16, tag="ones")
        nc.vector.memset(ones64[:], 1.0)
        dumm = st_pool.tile([P, 1], F32, tag="dumm")
        nc.vector.memset(dumm[:], 1.0)
        dumm2 = st_pool.tile([P, 1], F32, tag="dumm2")
        nc.scalar.activation(dumm2[:], dumm[:], AF.Sqrt)

        # PE warm-up while waiting for the first chunk (HAM -> K=8/8).
        pw = psW.tile([P, HB], F32, tag="pw")
        NWARM = 9
        for i in range(NWARM):
            nc.tensor.matmul(
                pw[:], wz[:, :P], wz[:],
                start=(i == 0), stop=(i == NWARM - 1),
            )

        po = [
            psO.tile([C, HB], F32, tag=f"ot{h}", name=f"po{h}")
            for h in range(2)
        ]
        pf = [
            psF.tile([C, HB], F32, tag=f"fs{h}", name=f"pf{h}")
            for h in range(2)
        ]
        for k in range(DC):
            chunk = fxt_sb[:, k * MS:(k + 1) * MS]
            # out.T slab: OT[c, m] += A_k[d, c].T fxt_k[d, m]. Chunk 0's
            # h0 matmul is split to match its quarter DMAs: the first
            # start=True matmul clears the whole bank, the second writes
            # its untouched columns (has_written semantics).
            if k == 0:
                nc.tensor.matmul(
                    po[0][:, 0:QB], a_sb[:, 0:C], chunk[:, 0:QB],
                    start=True, stop=False,
                )
                nc.tensor.matmul(
                    po[0][:, QB:HB], a_sb[:, 0:C], chunk[:, QB:HB],
                    start=False, stop=False,
                )
                nc.tensor.matmul(
                    po[1][:], a_sb[:, 0:C], chunk[:, HB:MS],
                    start=True, stop=False,
                )
            else:
                for h in range(2):
                    nc.tensor.matmul(
                        po[h][:],
                        a_sb[:, k * C:(k + 1) * C],
                        chunk[:, h * HB:(h + 1) * HB],
                        start=False,
                        stop=(k == DC - 1),
                    )
            # query-norm squares (tensor_tensor bf16 runs the 2x uop),
            # then ones-matmuls reduce over this chunk's dims into pf,
            # replicated across 64 output partitions to match po's layout.
            sqd = sq_pool.tile([P, MS], BF16, tag="sq")
            nc.vector.tensor_mul(sqd[:], chunk, chunk)
            for h in range(2):
                nc.tensor.matmul(
                    pf[h][:],
                    ones64[:],
                    sqd[:, h * HB:(h + 1) * HB],
                    start=(k == 0),
                    stop=(k == DC - 1),
                )

        # Tail per half, pipelined ACT->DVE->store: fnorm = sqrt(fsq),
        # 1/fnorm, scale the PSUM slab, store bf16 (host upcasts).
        for h in range(2):
            fnorm = nm_pool.tile([C, HB], F32, tag=f"fn{h}", name=f"fn{h}")
            nc.scalar.activation(fnorm[:], pf[h][:], AF.Sqrt)
            frinv = nm_pool.tile([C, HB], F32, tag=f"fr{h}", name=f"fr{h}")
            nc.vector.reciprocal_approx_fast(frinv[:], fnorm[:])
            os = os_pool.tile([C, HB], BF16, tag=f"os{h}", name=f"os{h}")
            nc.vector.tensor_mul(os[:], po[h][:], frinv[:])
            (nc.sync, nc.scalar)[h].dma_start(
                ot[:, h * HB:(h + 1) * HB], os[:]
            )

    nc.compile()
    return nc


def _get_ncs():
    if "nc1" not in _CACHE:
        _CACHE["nc1"] = _build_phase1()
        _CACHE["nc2"] = _build_phase2()
    return _CACHE["nc1"], _CACHE["nc2"]


class _FakeResult:
    def __init__(self, results):
        self.results = results
        self.exec_time_ns = None
        self.instructions_and_trace = None


def _make_runner(nc):
    """One persistently-jitted shard_map executable for this Bass module.

    run_bass_via_pjrt rebuilds its jit closure per call, which retraces and
    re-lowers the HLO every invocation (~3 s/launch of host time). Caching
    the jitted callable keeps warmed kernel() calls fast; the device-side
    NEFF and its execution are identical.
    """
    import jax
    import numpy as _np

    bass2jax.install_neuronx_cc_hook()
    Mesh = bass2jax.Mesh
    PartitionSpec = bass2jax.PartitionSpec
    shard_map = bass2jax.shard_map

    partition_name = (
        nc.partition_id_tensor.name if nc.partition_id_tensor else None
    )
    in_names, out_names, out_avals, zero_shapes = [], [], [], []
    for alloc in nc.m.functions[0].allocations:
        if not isinstance(alloc, mybir.MemoryLocationSet):
            continue
        name = alloc.memorylocations[0].name
        if alloc.kind == "ExternalInput":
            if name != partition_name:
                in_names.append(name)
        elif alloc.kind == "ExternalOutput":
            shape = tuple(alloc.tensor_shape)
            dtype = mybir.dt.np(alloc.dtype)
            out_avals.append(jax.core.ShapedArray(shape, dtype))
            out_names.append(name)
            zero_shapes.append((shape, dtype))
    n_params = len(in_names)
    all_in = list(in_names) + list(out_names)
    if partition_name is not None:
        all_in.append(partition_name)
    donate = tuple(range(n_params, n_params + len(out_names)))

    def _body(*args):
        operands = list(args)
        if partition_name is not None:
            operands.append(bass2jax.partition_id_tensor())
        outs = bass2jax._bass_exec_p.bind(
            *operands,
            out_avals=tuple(out_avals),
            in_names=tuple(all_in),
            out_names=tuple(out_names),
            lowering_input_output_aliases=(),
            sim_require_finite=True,
            sim_require_nnan=True,
            nc=nc,
        )
        return tuple(outs)

    devices = jax.devices()[:NCORES]
    mesh = Mesh(_np.asarray(devices), ("core",))
    nspec = n_params + len(out_names)
    sharded = jax.jit(
        shard_map(
            _body, mesh=mesh,
            in_specs=(PartitionSpec("core"),) * nspec,
            out_specs=(PartitionSpec("core"),) * len(out_names),
            check_rep=False,
        ),
        donate_argnums=donate,
        keep_unused=True,
    )

    def runner(in_maps):
        concat_in = [
            _np.concatenate([_np.asarray(m[name]) for m in in_maps], axis=0)
            for name in in_names
        ]
        concat_zeros = [
            _np.zeros((NCORES * s[0], *s[1:]), dt) for s, dt in zero_shapes
        ]
        out_arrs = sharded(*concat_in, *concat_zeros)
        return _FakeResult([
            {
                name: _np.asarray(out_arrs[i]).reshape(
                    NCORES, *out_avals[i].shape
                )[c]
                for i, name in enumerate(out_names)
            }
            for c in range(NCORES)
        ])

    return runner


def _get_runners():
    if "run1" not in _CACHE:
        nc1, nc2 = _get_ncs()
        _CACHE["run1"] = _make_runner(nc1)
        _CACHE["run2"] = _make_runner(nc2)
    return _CACHE["run1"], _CACHE["run2"]


def _tile_rows(arr, ntiles):
    """[ntiles*128, F] -> [128, ntiles, F] with [p, t, f] = arr[t*128+p, f]."""
    f = arr.shape[1]
    return np.ascontiguousarray(arr.reshape(ntiles, P, f).transpose(1, 0, 2))


def run(gS, fX, trainTarget, nClasses, trace=False, **spmd_kwargs):
    nc1, nc2 = _get_ncs()
    gS = np.asarray(gS, dtype=np.float32)
    fX = np.asarray(fX, dtype=np.float32)
    tt = np.asarray(trainTarget).astype(np.int64).ravel()
    nc_classes = int(np.asarray(nClasses))
    assert nc_classes == C and gS.shape == (N, D) and fX.shape == (M, D)

    oh = np.zeros((N, C), dtype=BF)
    oh[np.arange(N), tt] = 1.0
    gsb = gS.astype(BF)
    fxb = fX.astype(BF)

    in_maps1 = []
    for i in range(NCORES):
        in_maps1.append({
            "gs": _tile_rows(gsb[i * NS:(i + 1) * NS], NT),
            "oh": _tile_rows(oh[i * NS:(i + 1) * NS], NT),
        })
    if trace or spmd_kwargs:
        res1 = run_bass_kernel_spmd(
            nc1, in_maps1, core_ids=list(range(NCORES)), trace=trace,
            **spmd_kwargs
        )
    else:
        res1 = _get_runners()[0](in_maps1)
    # gather-reduce the partial A.T's [64, 1024], retile to [128, 8, 64]
    at = np.zeros((C, D), dtype=np.float32)
    for i in range(NCORES):
        at += res1.results[i]["atp"]
    a_tiled = np.ascontiguousarray(
        at.T.astype(BF).reshape(DC, P, C).transpose(1, 0, 2)
    )

    in_maps2 = []
    for i in range(NCORES):
        sl = fxb[i * MS:(i + 1) * MS]                     # [MS, D] bf16
        fxt_tiled = np.ascontiguousarray(
            sl.T.reshape(DC, P, MS).transpose(1, 0, 2)
        )
        in_maps2.append({"a": a_tiled, "fxt": fxt_tiled})
    if trace or spmd_kwargs:
        res2 = run_bass_kernel_spmd(
            nc2, in_maps2, core_ids=list(range(NCORES)), trace=trace,
            **spmd_kwargs
        )
    else:
        res2 = _get_runners()[1](in_maps2)
    full = np.empty((M, C), dtype=np.float32)
    for i in range(NCORES):
        full[i * MS:(i + 1) * MS] = res2.results[i]["ot"].T
    return full, (res1, res2)


def kernel(gS, fX, trainTarget, nClasses):
    full, _ = run(gS, fX, trainTarget, nClasses)
    return full
